# revision 26
# baseline (speedup 1.0000x reference)
"""Trainium2 Bass kernel for nn_BasicLSTM: (B,T,N,C) shared-weight LSTM -> FC.

Strategy (data parallel over 8 cores, B=64 -> 8 batches/core):
  - seqs = 8*1370 = 10960 independent (b,n) sequences per core, T=12, C=8, H=64.
  - Layout "gates on partitions, sequences on free dim". Per step t, per
    sequence-block pair (two blocks of S=512 seqs), 8 matmuls (4 gates x 2
    blocks) with stationary lhsT = [W_hh.T; W_ih.T; b] (73 x 64) compute gate
    pre-activations into gate-major PSUM: block0 -> partitions 0:64,
    block1 -> partitions 64:128 (tensor-engine col tile position 64).
  - ACT: one Sigmoid spanning the [i|f|o] PSUM banks (128,1536), one Tanh on
    the g bank, one Tanh on the cell state. DVE: i*g, f*c, add, o*tanh(c).
  - h is written by DVE directly into the next step's matmul rhs tile
    [h(0:64); x_t(64:72); ones(72)]; block1's h (partitions 64:128) moves to
    partitions 0:64 of its rhs via SBUF->SBUF DMA.
  - x arrives pre-transposed from the host as (9, T, seqs) bf16 (channel 8 is
    the constant 1.0 row that carries the biases through the contraction).
  - FC tail: y = W_fc @ h_T + b_fc as K=65 matmuls over the stored final h.
"""

import os
from contextlib import ExitStack

import numpy as np

import concourse.bass as bass
import concourse.mybir as mybir
import concourse.tile as tile
from concourse import bacc
from concourse.bass_utils import run_bass_kernel_spmd
from concourse.tile import add_dep_helper

B, T, N, C, H = 64, 12, 1370, 8, 64
NCORES = 8
BPC = B // NCORES          # batches per core
SEQS = BPC * N             # 10960 sequences per core
S = 512                    # sequence block size (free dim per matmul)
KH = H                     # 64 rows of h in rhs
KX = C + 1                 # 8 x-channels + ones row
K = KH + KX                # 73
G4 = 4 * H                 # 256

BF16 = mybir.dt.bfloat16
F32 = mybir.dt.float32
CDT = BF16  # cell-state dtype (bf16 doubles DVE throughput on the c update)
NPBF16 = mybir.dt.np(BF16)

AF = mybir.ActivationFunctionType


def _blocks(seqs: int, s: int):
    out = []
    c0 = 0
    while c0 < seqs:
        out.append((c0, min(s, seqs - c0)))
        c0 += s
    return out


def _fence(nc, producers):
    """The TRN2 Matmult ISA slot holds a single sync-wait, but the first
    matmul of a step naturally depends on 3-4 engines (ACT psum release, DVE
    h-write, DMA x / h-move). Funnel those deps through a chain of SyncE nops
    (one wait each); callers then depend only on the returned nop."""
    last = None
    seen = set()
    for p in producers:
        if p is None:
            continue
        pi = p.ins if hasattr(p, "ins") else p
        if id(pi) in seen:
            continue
        seen.add(id(pi))
        n = nc.sync.nop(nofuse=True, hint="depfence")
        add_dep_helper(n.ins, pi, reason="dep fence")
        last = n
    return last


def build_nc(seqs: int = SEQS, s: int = S, use_fence: bool = False) -> bass.Bass:
    nc = bacc.Bacc("TRN2", target_bir_lowering=False, debug=False)

    xin = nc.declare_dram_parameter("xin", [KX, T, seqs], BF16, isOutput=False)
    wg = nc.declare_dram_parameter("wg", [K, G4], BF16, isOutput=False)
    wfc = nc.declare_dram_parameter("wfc", [H + 1, C], BF16, isOutput=False)
    y = nc.declare_dram_parameter("y", [C, seqs], F32, isOutput=True)

    blocks = _blocks(seqs, s)
    pairs = [blocks[i : i + 2] for i in range(0, len(blocks), 2)]

    with tile.TileContext(nc) as tc, ExitStack() as ctx:
        const = ctx.enter_context(tc.tile_pool(name="const", bufs=1))
        rhsp = ctx.enter_context(tc.tile_pool(name="rhs", bufs=84))
        sgp = ctx.enter_context(tc.tile_pool(name="sg", bufs=8))
        thgp = ctx.enter_context(tc.tile_pool(name="thg", bufs=8))
        thcp = ctx.enter_context(tc.tile_pool(name="thc", bufs=8))
        igp = ctx.enter_context(tc.tile_pool(name="ig", bufs=6))
        fcpool = ctx.enter_context(tc.tile_pool(name="fcp", bufs=6))
        cpool = ctx.enter_context(tc.tile_pool(name="cst", bufs=8))
        htp = ctx.enter_context(tc.tile_pool(name="htmp", bufs=8))
        ysp = ctx.enter_context(tc.tile_pool(name="ys", bufs=4))
        pifo = ctx.enter_context(tc.tile_pool(name="pifo", bufs=2, space="PSUM"))
        pgp = ctx.enter_context(tc.tile_pool(name="pg", bufs=2, space="PSUM"))

        w_sb = const.tile([K, G4], BF16)
        nc.sync.dma_start(out=w_sb[:, :], in_=wg[:, :])
        wfc_sb = const.tile([H + 1, C], BF16)
        nc.sync.dma_start(out=wfc_sb[:, :], in_=wfc[:, :])
        hstore = const.tile([H + 1, seqs], BF16)
        # ones row comes from xin's ones channel (avoids a slow gpsimd memset
        # that would delay the x-prefetch DMA queue)
        nc.sync.dma_start(out=hstore[H : H + 1, :], in_=xin[C : C + 1, 0, :])
        # PE warm-up + early ACT table load, all under the initial DMA shadow
        scratch = const.tile([128, s], BF16)
        nc.vector.memset(scratch[:, :], 1.0)
        nc.scalar.activation(scratch[0:1, 0:8], scratch[0:1, 0:8], AF.Sigmoid)

        # gate column offsets in w_sb: pytorch order i, f, g, o
        WI, WF, WG, WO = 0, H, 2 * H, 3 * H

        def emit_step(st, t):
            pair, rhs_t, c_t = st["pair"], st["rhs"], st["c"]
            ifo = pifo.tile([128, 3 * s], F32, name="ifo", tag="ifo")
            pg = pgp.tile([128, s], F32, name="pg", tag="pg")

            # matmuls; g gate first so its tanh can start early
            for gof, dst, dsec in (
                (WG, pg, 0),
                (WI, ifo, 0),
                (WF, ifo, 1),
                (WO, ifo, 2),
            ):
                for blk, (c0, wd) in enumerate(pair):
                    pb = 64 * blk
                    if t == 0:
                        lh = w_sb[KH:K, gof : gof + H]
                        rh = rhs_t[t][blk][KH:K, 0:wd]
                    else:
                        lh = w_sb[:, gof : gof + H]
                        rh = rhs_t[t][blk][:, 0:wd]
                    nc.tensor.matmul(
                        dst[pb : pb + 64, dsec * s : dsec * s + wd], lh, rh
                    )

            sg = sgp.tile([128, 3 * s], BF16, name="sg", tag="sg")
            nc.scalar.activation(sg[:, :], ifo[:, :], AF.Sigmoid)
            thg = thgp.tile([128, s], BF16, name="thg", tag="thg")
            nc.scalar.activation(thg[:, :], pg[:, :], AF.Tanh)

            if t == 0:
                nc.vector.tensor_mul(c_t[:, :], sg[:, 0:s], thg[:, :])
            else:
                ig = igp.tile([128, s], BF16, name="ig", tag="ig")
                nc.vector.tensor_mul(ig[:, :], sg[:, 0:s], thg[:, :])
                fc = fcpool.tile([128, s], CDT, name="fc", tag="fc")
                nc.vector.tensor_mul(fc[:, :], sg[:, s : 2 * s], c_t[:, :])
                nc.vector.tensor_add(c_t[:, :], ig[:, :], fc[:, :])

            thc = thcp.tile([128, s], BF16, name="thc", tag="thc")
            nc.scalar.activation(thc[:, :], c_t[:, :], AF.Tanh)

            # h = sigmoid(o) * tanh(c)
            for blk, (c0, wd) in enumerate(pair):
                pb = 64 * blk
                so = sg[pb : pb + 64, 2 * s : 2 * s + wd]
                tc_half = thc[pb : pb + 64, 0:wd]
                if t == T - 1:
                    dst = hstore[0:H, c0 : c0 + wd]
                else:
                    dst = rhs_t[t + 1][blk][0:KH, 0:wd]
                if blk == 0:
                    nc.vector.tensor_mul(dst, so, tc_half)
                else:
                    ht = htp.tile([128, s], BF16, name="ht", tag="ht")
                    nc.vector.tensor_mul(ht[pb : pb + 64, 0:wd], so, tc_half)
                    nc.sync.dma_start(out=dst, in_=ht[pb : pb + 64, 0:wd])

        # process pairs in interleaved groups so several independent
        # recurrences keep every engine busy (and TensorE HAM-warm)
        GROUP = 8
        PREFETCH = 3

        def alloc_rhs(st, t):
            pair, rhs_t = st["pair"], st["rhs"]
            rhs_t[t] = [
                rhsp.tile([K, s], BF16, name="rhs", tag="rhs")
                for _ in range(len(pair))
            ]
            for blk, (c0, wd) in enumerate(pair):
                nc.gpsimd.dma_start(
                    out=rhs_t[t][blk][KH:K, 0:wd],
                    in_=xin[:, t, c0 : c0 + wd],
                )

        for g0 in range(0, len(pairs), GROUP):
            grp = pairs[g0 : g0 + GROUP]
            states = []
            for pair in grp:
                c_t = cpool.tile([128, s], CDT, name="c_t", tag="c_t")
                states.append({"pair": pair, "rhs": [None] * T, "c": c_t})
                for t in range(PREFETCH):
                    alloc_rhs(states[-1], t)
            for t in range(T):
                for st in states:
                    if t + PREFETCH < T:
                        alloc_rhs(st, t + PREFETCH)
                    emit_step(st, t)
            # FC for this group's blocks; overlaps the next group's ramp-up
            for st in states:
                for c0, wd in st["pair"]:
                    pf = pgp.tile([128, s], F32, tag="pg", name="pf")
                    nc.tensor.matmul(
                        pf[0:C, 0:wd], wfc_sb[:, :], hstore[:, c0 : c0 + wd]
                    )
                    yt = ysp.tile([C, s], F32, name="yt", tag="yt")
                    nc.vector.tensor_copy(yt[:, 0:wd], pf[0:C, 0:wd])
                    nc.sync.dma_start(out=y[:, c0 : c0 + wd], in_=yt[:, 0:wd])


    nc.compile()
    return nc


def prep_inputs(x, W_ih, W_hh, b_ih, b_hh, W_fc, b_fc, seqs=SEQS, ncores=NCORES):
    """Host-side shard + transpose + weight packing. Returns in_maps."""
    x = np.asarray(x, dtype=np.float32)
    W_ih = np.asarray(W_ih, dtype=np.float32)
    W_hh = np.asarray(W_hh, dtype=np.float32)
    b = np.asarray(b_ih, dtype=np.float32) + np.asarray(b_hh, dtype=np.float32)
    W_fc = np.asarray(W_fc, dtype=np.float32)
    b_fc = np.asarray(b_fc, dtype=np.float32)

    wg = np.zeros((K, G4), dtype=np.float32)
    for g in range(4):
        rows = slice(H * g, H * g + H)
        wg[0:KH, H * g : H * g + H] = W_hh[rows, :].T
        wg[KH : KH + C, H * g : H * g + H] = W_ih[rows, :].T
        wg[K - 1, H * g : H * g + H] = b[rows]
    wg = wg.astype(NPBF16)

    wfc = np.concatenate([W_fc.T, b_fc[None, :]], axis=0).astype(NPBF16)  # (65, 8)

    bpc = x.shape[0] // ncores
    in_maps = []
    for k in range(ncores):
        xc = x[k * bpc : (k + 1) * bpc]              # (bpc, T, N, C)
        xt = xc.transpose(3, 1, 0, 2).reshape(C, T, seqs)
        xext = np.empty((KX, T, seqs), dtype=NPBF16)
        xext[0:C] = xt.astype(NPBF16)
        xext[C] = np.ones((T, seqs), dtype=NPBF16)
        in_maps.append({"xin": xext, "wg": wg, "wfc": wfc})
    return in_maps


_CACHE = {}


def _get_nc():
    if "nc" not in _CACHE:
        _CACHE["nc"] = build_nc()
    return _CACHE["nc"]


def kernel(x, W_ih, W_hh, b_ih, b_hh, W_fc, b_fc, **run_kwargs):
    nc = _get_nc()
    in_maps = prep_inputs(x, W_ih, W_hh, b_ih, b_hh, W_fc, b_fc)
    res = run_bass_kernel_spmd(nc, in_maps, list(range(NCORES)), **run_kwargs)
    outs = res.results
    ys = []
    for k in range(NCORES):
        yk = np.asarray(outs[k]["y"])               # (C, SEQS) f32
        ys.append(yk.T.reshape(BPC, N, C))
    y = np.concatenate(ys, axis=0)                  # (B, N, C)
    if run_kwargs.get("trace"):
        _CACHE["last_result"] = res
    return y.astype(np.float32)
